# revision 8
# baseline (speedup 1.0000x reference)
"""Trainium2 Bass kernel for nn_Aggregation (sparse local attention aggregation).

out[n, g*64+cw, y, x] = sum_{i,j in 3x3} input[n, g*64+cw, y+i-1, x+j-1] * weight[n, cw, i*3+j, y*64+x]

Sharding: data-parallel over batch n: 8 cores x 2 batches each.
Per-core partition layout: p = b*64 + cw  (2 local batches x 64 weight channels).
The 8 groups live in the free dimension; the weight is broadcast over groups
with a stride-0 access pattern (no duplication needed).

Engine plan:
  - DMA: stream input/weight/output bands (128-partition transfers).
  - ScalarE (ACT): cast fp32 -> compute dtype, scattering into a zero-padded
    (66-wide rows) plane; also evacuates PSUM -> SBUF.
  - VectorE (DVE): 9 shifted tensor-tensor multiplies per band.
  - TensorE (PE): accumulates the 9 product planes via identity matmuls
    into PSUM (fp32).
"""

import os

import numpy as np

N, C, H, W = 16, 512, 64, 64
CW, G, K = 64, 8, 3
NCORE = 8
NB = N // NCORE          # batches per core
L = H * W

# tunables
R = 8                    # band rows (R*64 = 512 = one PSUM bank per group)
RP = R + 2               # plane rows incl. halo
NBANDS = H // R
WPLANE = W + 2           # 66
MODE = os.environ.get("AGG_KERNEL_MODE", "fp16")  # "fp32" | "fp16" | "bf16"

_cache = {}


def _build(mode):
    import concourse.mybir as mybir
    from concourse import bacc
    from concourse.tile import TileContext
    from concourse.masks import make_identity

    cdt = {
        "fp32": mybir.dt.float32,
        "fp16": mybir.dt.float16,
        "bf16": mybir.dt.bfloat16,
    }[mode]
    f32 = mybir.dt.float32

    nc = bacc.Bacc()
    x_t = nc.dram_tensor("input", [NB, C, H, W], f32, kind="ExternalInput")
    w_t = nc.dram_tensor("weight", [NB, CW, K * K, L], f32, kind="ExternalInput")
    o_t = nc.dram_tensor("out", [NB, C, H, W], f32, kind="ExternalOutput")

    # DRAM views
    xv = x_t.rearrange("b (g cw) h w -> b cw g (h w)", g=G)   # [2, 64, 8, 4096]
    ov = o_t.rearrange("b (g cw) h w -> b cw g (h w)", g=G)
    wv = w_t.rearrange("b cw k l -> (b cw) k l")              # [128, 9, 4096]

    PL = G * RP * WPLANE            # padded plane length per partition

    with TileContext(nc) as tc:
        with (
            tc.tile_pool(name="const", bufs=1) as const_pool,
            tc.tile_pool(name="xs", bufs=2) as xs_pool,
            tc.tile_pool(name="xe", bufs=2) as xe_pool,
            tc.tile_pool(name="wt", bufs=2) as wt_pool,
            tc.tile_pool(name="pr", bufs=2) as pr_pool,
            tc.tile_pool(name="os", bufs=2) as os_pool,
            tc.tile_pool(name="ps", bufs=8, space="PSUM") as ps_pool,
        ):
            ident = const_pool.tile([128, 128], cdt)
            make_identity(nc, ident)

            use_odd = mode != "fp32"

            for band in range(NBANDS):
                y0 = band * R
                row_lo = max(y0 - 1, 0)             # first loaded image row
                row_hi = min(y0 + R + 1, H)         # one past last loaded row
                RL = row_hi - row_lo                # rows loaded
                prow0 = 0 if y0 > 0 else 1          # plane row of first loaded row

                # ---- load x band (one DMA per group, 128 partitions each)
                xs = xs_pool.tile([128, G, RP * W], f32, tag="xs")
                for g in range(G):
                    nc.sync.dma_start(
                        out=xs[:, g, : RL * W],
                        in_=xv[:, :, g, row_lo * W : row_hi * W],
                    )

                # ---- padded planes (cast + scatter)
                xe = xe_pool.tile([128, PL + 4 * WPLANE], cdt, tag="xe")
                planes = [xe]
                if use_odd:
                    xo = xe_pool.tile([128, PL + 4 * WPLANE], cdt, tag="xo")
                    planes.append(xo)

                xev = xe[:, :PL].rearrange("p (g rc) -> p g rc", g=G)
                # zero the column pads: pairs (65, 66) every 66 elems
                nc.gpsimd.memset(
                    xe[:, W + 1 : W + 1 + 66 * G * RP].rearrange(
                        "p (n c) -> p n c", c=WPLANE
                    )[:, :, 0:2],
                    0.0,
                )
                nc.gpsimd.memset(xe[:, 0:1], 0.0)
                if use_odd:
                    # x_odd needs no column pads (only cols 0..63 are read),
                    # but needs halo rows for edge bands (memset below).
                    pass
                # halo rows for edge bands
                for pl in planes:
                    plv = pl[:, :PL].rearrange("p (g rc) -> p g rc", g=G)
                    if band == 0:
                        nc.gpsimd.memset(plv[:, :, 0:WPLANE], 0.0)
                    if band == NBANDS - 1:
                        nc.gpsimd.memset(
                            plv[:, :, (RP - 1) * WPLANE : RP * WPLANE], 0.0
                        )

                # cast/scatter loaded rows into plane rows [prow0, prow0+RL)
                src = xs[:, :, : RL * W].rearrange("p g (r c) -> p g r c", c=W)
                dst_e = xev[
                    :, :, prow0 * WPLANE : prow0 * WPLANE + RL * WPLANE
                ].rearrange("p g (r c) -> p g r c", c=WPLANE)[:, :, :, 1 : W + 1]
                nc.scalar.copy(out=dst_e, in_=src)
                if use_odd:
                    xov = xo[:, :PL].rearrange("p (g rc) -> p g rc", g=G)
                    dst_o = xov[
                        :, :, prow0 * WPLANE : prow0 * WPLANE + RL * WPLANE
                    ].rearrange("p g (r c) -> p g r c", c=WPLANE)[:, :, :, 0:W]
                    nc.scalar.copy(out=dst_o, in_=src)

                # ---- load weight band (single DMA, 128 partitions)
                wt = wt_pool.tile([128, K * K, R * W], f32, tag="wt")
                nc.sync.dma_start(
                    out=wt[:], in_=wv[:, :, y0 * W : (y0 + R) * W]
                )
                if mode == "fp32":
                    wc = wt
                else:
                    wc = wt_pool.tile([128, K * K, R * W], cdt, tag="wc")
                    nc.scalar.copy(out=wc[:], in_=wt[:])

                # ---- products + PE accumulation
                ps_tiles = [
                    ps_pool.tile([128, R * W], f32, tag="ps", name=f"ps_{band}_{g}")
                    for g in range(G)
                ]
                for ij in range(K * K):
                    di, dj = ij // K - 1, ij % K - 1
                    # choose plane/offset so reads stay 4-byte aligned in fp16
                    if use_odd and dj == 0:
                        plane, col0 = planes[1], 0
                    else:
                        plane, col0 = planes[0], 1 + dj
                    off = (1 + di) * WPLANE + col0
                    xsrc = (
                        plane[:, off : off + PL]
                        .rearrange("p (g rc) -> p g rc", g=G)[:, :, : R * WPLANE]
                        .rearrange("p g (r c) -> p g r c", c=WPLANE)[:, :, :, 0:W]
                    )
                    wsrc = (
                        wc[:, ij, :]
                        .rearrange("p (r c) -> p r c", c=W)
                        .unsqueeze(1)
                        .broadcast_to([128, G, R, W])
                    )
                    pr = pr_pool.tile([128, G, R * W], cdt, tag="pr")
                    prv = pr.rearrange("p g (r c) -> p g r c", c=W)
                    nc.vector.tensor_mul(out=prv, in0=xsrc, in1=wsrc)
                    for g in range(G):
                        nc.tensor.matmul(
                            ps_tiles[g],
                            ident,
                            pr[:, g, :],
                            start=(ij == 0),
                            stop=(ij == K * K - 1),
                        )

                # ---- evacuate PSUM and store
                os = os_pool.tile([128, G, R * W], f32, tag="os")
                for g in range(G):
                    nc.scalar.copy(out=os[:, g, :], in_=ps_tiles[g][:])
                for g in range(G):
                    nc.sync.dma_start(
                        out=ov[:, :, g, y0 * W : (y0 + R) * W], in_=os[:, g, :]
                    )

    nc.finalize()
    return nc


def _get(mode):
    if mode not in _cache:
        _cache[mode] = _build(mode)
    return _cache[mode]


def kernel(input: np.ndarray, weight: np.ndarray) -> np.ndarray:
    from concourse.bass_utils import run_bass_kernel_spmd

    input = np.ascontiguousarray(input, dtype=np.float32)
    weight = np.ascontiguousarray(weight, dtype=np.float32)
    nc = _get(MODE)
    in_maps = [
        {
            "input": input[i * NB : (i + 1) * NB],
            "weight": weight[i * NB : (i + 1) * NB],
        }
        for i in range(NCORE)
    ]
    res = run_bass_kernel_spmd(nc, in_maps, core_ids=list(range(NCORE)))
    return np.concatenate([res.results[i]["out"] for i in range(NCORE)], axis=0)


# revision 14
# speedup vs baseline: 4.0692x; 4.0692x over previous
"""Trainium2 Bass kernel for nn_Aggregation (sparse local attention aggregation).

out[n, g*64+cw, y, x] = sum_{i,j in 3x3} input[n, g*64+cw, y+i-1, x+j-1]
                        * weight[n, cw, i*3+j, y*64+x]

Sharding: data-parallel over batch n: 8 cores x 2 batches each.

Per-core layouts (host pre-swizzled so every DMA is a 2-dim
[128 partitions x contiguous] transfer, which sprays across all 16 SDMA
engines; 3-dim APs land on only 2 engines):
  input_t : [128=(b,cw), H, (g,x)]    partition p = b*64 + cw
  weight_t: [128=(b,cw), H, (ij,x)]
  out_t   : [128=(b,cw), H, (g,x)]

The 8 groups live in the free dimension; the weight is broadcast over
groups with a stride-0 access pattern (no duplication).

Engines:
  DMA   : band loads/stores; fp32->fp16 cast happens in the DMA (SWDGE).
  ACT   : scatters rows into zero-padded (66-wide) even/odd planes,
          evacuates PSUM->SBUF.
  DVE   : 9 shifted tensor-tensor multiplies per band (fp16 2x mode; the
          even/odd planes keep every read 4-byte aligned).
  PE    : accumulates the 9 product planes into PSUM via identity matmuls.
"""

import os

import numpy as np

N, C, H, W = 16, 512, 64, 64
CW, G, K = 64, 8, 3
NCORE = 8
NB = N // NCORE          # batches per core
L = H * W

R = 8                    # band rows (one PSUM bank per output row)
RP = R + 2               # plane rows incl. halo
NBANDS = H // R
WP = W + 2               # 66
GWP = G * WP             # one padded row-block (all groups)
MODE = os.environ.get("AGG_KERNEL_MODE", "fp16")  # "fp32" | "fp16" | "bf16"

_cache = {}


def _build(mode):
    import concourse.mybir as mybir
    from concourse import bacc
    from concourse.tile import TileContext
    from concourse.masks import make_identity

    cdt = {
        "fp32": mybir.dt.float32,
        "fp16": mybir.dt.float16,
        "bf16": mybir.dt.bfloat16,
    }[mode]
    f32 = mybir.dt.float32

    nc = bacc.Bacc()
    x_t = nc.dram_tensor("input_t", [128, H, G * W], f32, kind="ExternalInput")
    w_t = nc.dram_tensor("weight_t", [128, H, K * K * W], f32, kind="ExternalInput")
    o_t = nc.dram_tensor("out_t", [128, H, G * W], f32, kind="ExternalOutput")

    PL = RP * GWP            # padded plane length per partition

    with TileContext(nc) as tc:
        with (
            tc.tile_pool(name="const", bufs=1) as const_pool,
            tc.tile_pool(name="xs", bufs=2) as xs_pool,
            tc.tile_pool(name="xe", bufs=2) as xe_pool,
            tc.tile_pool(name="wt", bufs=2) as wt_pool,
            tc.tile_pool(name="pr", bufs=2) as pr_pool,
            tc.tile_pool(name="os", bufs=1) as os_pool,
            tc.tile_pool(name="ps", bufs=8, space="PSUM") as ps_pool,
        ):
            ident = const_pool.tile([128, 128], cdt)
            make_identity(nc, ident)

            use_odd = mode != "fp32"

            for band in range(NBANDS):
                y0 = band * R
                row_lo = max(y0 - 1, 0)             # first loaded image row
                row_hi = min(y0 + R + 1, H)         # one past last loaded row
                RL = row_hi - row_lo                # rows loaded
                prow0 = 0 if y0 > 0 else 1          # plane row of first loaded row

                # ---- load x band: one 2-dim DMA (sprays all 16 engines)
                xs = xs_pool.tile([128, RP * G * W], f32, tag="xs")
                nc.sync.dma_start(
                    out=xs[:, : RL * G * W], in_=x_t[:, row_lo:row_hi, :]
                )

                # ---- padded even/odd planes
                xe = xe_pool.tile([128, PL + 66], cdt, tag="xe")
                planes = [xe]
                if use_odd:
                    xo = xe_pool.tile([128, PL + 66], cdt, tag="xo")
                    planes.append(xo)

                # zero the xe column pads: pairs (65, 66) every 66 elems
                nc.gpsimd.memset(
                    xe[:, W + 1 : W + 1 + WP * G * RP].rearrange(
                        "p (n c) -> p n c", c=WP
                    )[:, :, 0:2],
                    0.0,
                )
                nc.gpsimd.memset(xe[:, 0:1], 0.0)
                # halo rows for edge bands
                for pl in planes:
                    if band == 0:
                        nc.gpsimd.memset(pl[:, 0:GWP], 0.0)
                    if band == NBANDS - 1:
                        nc.gpsimd.memset(pl[:, (RP - 1) * GWP : RP * GWP], 0.0)

                # scatter loaded rows into plane rows [prow0, prow0+RL)
                src = xs[:, : RL * G * W].rearrange(
                    "p (r g c) -> p r g c", g=G, c=W
                )
                dst_e = (
                    xe[:, prow0 * GWP : (prow0 + RL) * GWP]
                    .rearrange("p (r g c) -> p r g c", g=G, c=WP)[:, :, :, 1 : W + 1]
                )
                nc.scalar.copy(out=dst_e, in_=src)
                if use_odd:
                    dst_o = (
                        xo[:, prow0 * GWP : (prow0 + RL) * GWP]
                        .rearrange("p (r g c) -> p r g c", g=G, c=WP)[:, :, :, 0:W]
                    )
                    nc.scalar.copy(out=dst_o, in_=src)

                # ---- load weight band: one 2-dim DMA (+ ACT cast if fp16)
                WROW = K * K * W                    # 576
                wt = wt_pool.tile([128, R * WROW + WROW], f32, tag="wt")
                nc.sync.dma_start(
                    out=wt[:, : R * WROW], in_=w_t[:, y0 : y0 + R, :]
                )
                if mode == "fp32":
                    wc = wt
                else:
                    wc = wt_pool.tile([128, R * WROW + WROW], cdt, tag="wc")
                    nc.scalar.copy(
                        out=wc[:, : R * WROW], in_=wt[:, : R * WROW]
                    )

                # ---- products + PE accumulation
                ps_tiles = [
                    ps_pool.tile([128, G * W], f32, tag="ps", name=f"ps_{band}_{r}")
                    for r in range(R)
                ]
                for ij in range(K * K):
                    di, dj = ij // K - 1, ij % K - 1
                    if use_odd and dj == 0:
                        plane, col0 = planes[1], 0
                    else:
                        plane, col0 = planes[0], 1 + dj
                    off = (1 + di) * GWP + col0
                    xsrc = (
                        plane[:, off : off + R * GWP]
                        .rearrange("p (r gc) -> p r gc", gc=GWP)
                        .rearrange("p r (g c) -> p r g c", c=WP)[:, :, :, 0:W]
                    )
                    wsrc = (
                        wc[:, ij * W : ij * W + R * WROW]
                        .rearrange("p (r s) -> p r s", s=WROW)[:, :, 0:W]
                        .unsqueeze(2)
                        .broadcast_to([128, R, G, W])
                    )
                    pr = pr_pool.tile([128, R * G * W], cdt, tag="pr")
                    prv = pr.rearrange("p (r g c) -> p r g c", g=G, c=W)
                    nc.vector.tensor_mul(out=prv, in0=xsrc, in1=wsrc)
                    for r in range(R):
                        nc.tensor.matmul(
                            ps_tiles[r],
                            ident,
                            pr[:, r * G * W : (r + 1) * G * W],
                            start=(ij == 0),
                            stop=(ij == K * K - 1),
                        )

                # ---- evacuate PSUM and store (one 2-dim DMA)
                os_ = os_pool.tile([128, R * G * W], f32, tag="os")
                for r in range(R):
                    nc.scalar.copy(
                        out=os_[:, r * G * W : (r + 1) * G * W], in_=ps_tiles[r]
                    )
                nc.sync.dma_start(out=o_t[:, y0 : y0 + R, :], in_=os_[:])

    nc.finalize()
    return nc


def _get(mode):
    if mode not in _cache:
        _cache[mode] = _build(mode)
    return _cache[mode]


def _swizzle_core(inp, wgt):
    # inp [2, 512, 64, 64] -> [128, H, G*W];  p = b*64+cw, free = (y, g, x)
    a = inp.reshape(NB, G, CW, H, W).transpose(0, 2, 3, 1, 4)
    a = np.ascontiguousarray(a).reshape(128, H, G * W)
    # wgt [2, 64, 9, 4096] -> [128, H, 9*W]; free = (y, ij, x)
    b = wgt.reshape(NB, CW, K * K, H, W).transpose(0, 1, 3, 2, 4)
    b = np.ascontiguousarray(b).reshape(128, H, K * K * W)
    return a, b


def _unswizzle_core(o):
    # [128, H, G*W] -> [2, 512, 64, 64]
    a = o.reshape(NB, CW, H, G, W).transpose(0, 3, 1, 2, 4)
    return np.ascontiguousarray(a).reshape(NB, C, H, W)


def kernel(input: np.ndarray, weight: np.ndarray) -> np.ndarray:
    from concourse.bass_utils import run_bass_kernel_spmd

    input = np.ascontiguousarray(input, dtype=np.float32)
    weight = np.ascontiguousarray(weight, dtype=np.float32)
    nc = _get(MODE)
    in_maps = []
    for i in range(NCORE):
        a, b = _swizzle_core(
            input[i * NB : (i + 1) * NB], weight[i * NB : (i + 1) * NB]
        )
        in_maps.append({"input_t": a, "weight_t": b})
    res = run_bass_kernel_spmd(nc, in_maps, core_ids=list(range(NCORE)))
    return np.concatenate(
        [_unswizzle_core(res.results[i]["out_t"]) for i in range(NCORE)], axis=0
    )


# revision 20
# speedup vs baseline: 4.1302x; 1.0150x over previous
"""Trainium2 Bass kernel for nn_Aggregation (sparse local attention aggregation).

out[n, g*64+cw, y, x] = sum_{i,j in 3x3} input[n, g*64+cw, y+i-1, x+j-1]
                        * weight[n, cw, i*3+j, y*64+x]

Sharding: data-parallel over batch n: 8 cores x 2 batches each.

Per-core layouts (host pre-swizzled so every DMA is a 2-dim
[128 partitions x contiguous] transfer, which sprays across all 16 SDMA
engines; 3-dim APs land on only 2 engines):
  input_t : [128=(b,cw), H, (g,x)]    partition p = b*64 + cw
  weight_t: [128=(b,cw), H, (ij,x)]
  out_t   : [128=(b,cw), H, (g,x)]

The 8 groups live in the free dimension; the weight is broadcast over
groups with a stride-0 access pattern (no duplication).

Engines:
  DMA   : band loads/stores; fp32->fp16 cast happens in the DMA (SWDGE).
  ACT   : scatters rows into zero-padded (66-wide) even/odd planes,
          evacuates PSUM->SBUF.
  DVE   : 9 shifted tensor-tensor multiplies per band (fp16 2x mode; the
          even/odd planes keep every read 4-byte aligned).
  PE    : accumulates the 9 product planes into PSUM via identity matmuls.
"""

import os

import numpy as np

N, C, H, W = 16, 512, 64, 64
CW, G, K = 64, 8, 3
NCORE = 8
NB = N // NCORE          # batches per core
L = H * W

R = 8                    # band rows (one PSUM bank per output row)
RP = R + 2               # plane rows incl. halo
NBANDS = H // R
WP = W + 2               # 66
GWP = G * WP             # one padded row-block (all groups)
MODE = os.environ.get("AGG_KERNEL_MODE", "fp16")  # "fp32" | "fp16" | "bf16"

_cache = {}


def _build(mode):
    import concourse.mybir as mybir
    from concourse import bacc
    from concourse.tile import TileContext
    from concourse.masks import make_identity

    cdt = {
        "fp32": mybir.dt.float32,
        "fp16": mybir.dt.float16,
        "bf16": mybir.dt.bfloat16,
    }[mode]
    f32 = mybir.dt.float32

    nc = bacc.Bacc()
    x_t = nc.dram_tensor("input_t", [128, H, G * W], f32, kind="ExternalInput")
    w_t = nc.dram_tensor("weight_t", [128, H, K * K * W], f32, kind="ExternalInput")
    o_t = nc.dram_tensor("out_t", [128, H, G * W], f32, kind="ExternalOutput")

    PL = RP * GWP            # padded plane length per partition

    with TileContext(nc) as tc:
        with (
            tc.tile_pool(name="const", bufs=1) as const_pool,
            tc.tile_pool(name="xs", bufs=2) as xs_pool,
            tc.tile_pool(name="xe", bufs=2) as xe_pool,
            tc.tile_pool(name="wt", bufs=2) as wt_pool,
            tc.tile_pool(name="pr", bufs=2) as pr_pool,
            tc.tile_pool(name="os", bufs=1) as os_pool,
            tc.tile_pool(name="ps", bufs=4, space="PSUM") as ps_pool,
        ):
            # Two identity copies: alternating the stationary operand lets
            # each LDWEIGHTS target the background weight buffer and overlap
            # the in-flight matmul (same-tensor LDW serializes instead).
            ident = const_pool.tile([128, 128], cdt)
            make_identity(nc, ident)
            ident2 = const_pool.tile([128, 128], cdt)
            make_identity(nc, ident2)
            idents = [ident, ident2]

            use_odd = mode != "fp32"

            for band in range(NBANDS):
                y0 = band * R
                row_lo = max(y0 - 1, 0)             # first loaded image row
                row_hi = min(y0 + R + 1, H)         # one past last loaded row
                RL = row_hi - row_lo                # rows loaded
                prow0 = 0 if y0 > 0 else 1          # plane row of first loaded row

                # ---- load x band: one 2-dim DMA (sprays all 16 engines)
                xs = xs_pool.tile([128, RP * G * W], f32, tag="xs")
                nc.sync.dma_start(
                    out=xs[:, : RL * G * W], in_=x_t[:, row_lo:row_hi, :]
                )

                # ---- padded even/odd planes
                xe = xe_pool.tile([128, PL + 66], cdt, tag="xe")
                planes = [xe]
                if use_odd:
                    xo = xe_pool.tile([128, PL + 66], cdt, tag="xo")
                    planes.append(xo)

                # zero the xe column pads: pairs (65, 66) every 66 elems
                nc.gpsimd.memset(
                    xe[:, W + 1 : W + 1 + WP * G * RP].rearrange(
                        "p (n c) -> p n c", c=WP
                    )[:, :, 0:2],
                    0.0,
                )
                nc.gpsimd.memset(xe[:, 0:1], 0.0)
                # halo rows for edge bands
                for pl in planes:
                    if band == 0:
                        nc.gpsimd.memset(pl[:, 0:GWP], 0.0)
                    if band == NBANDS - 1:
                        nc.gpsimd.memset(pl[:, (RP - 1) * GWP : RP * GWP], 0.0)

                # scatter loaded rows into plane rows [prow0, prow0+RL)
                src = xs[:, : RL * G * W].rearrange(
                    "p (r g c) -> p r g c", g=G, c=W
                )
                dst_e = (
                    xe[:, prow0 * GWP : (prow0 + RL) * GWP]
                    .rearrange("p (r g c) -> p r g c", g=G, c=WP)[:, :, :, 1 : W + 1]
                )
                nc.scalar.copy(out=dst_e, in_=src)
                if use_odd:
                    dst_o = (
                        xo[:, prow0 * GWP : (prow0 + RL) * GWP]
                        .rearrange("p (r g c) -> p r g c", g=G, c=WP)[:, :, :, 0:W]
                    )
                    nc.scalar.copy(out=dst_o, in_=src)

                # ---- load weight band: one 2-dim DMA (+ ACT cast if fp16)
                WROW = K * K * W                    # 576
                wt = wt_pool.tile([128, R * WROW + WROW], f32, tag="wt")
                nc.sync.dma_start(
                    out=wt[:, : R * WROW], in_=w_t[:, y0 : y0 + R, :]
                )
                if mode == "fp32":
                    wc = wt
                else:
                    wc = wt_pool.tile([128, R * WROW + WROW], cdt, tag="wc")
                    nc.scalar.copy(
                        out=wc[:, : R * WROW], in_=wt[:, : R * WROW]
                    )

                # ---- products + PE accumulation
                # 2 PSUM tiles of 4 banks each; each matmul writes one bank
                ps_tiles = [
                    ps_pool.tile(
                        [128, 2 * G * W], f32, tag="ps", name=f"ps_{band}_{t}"
                    )
                    for t in range(4)
                ]
                for ij in range(K * K):
                    di, dj = ij // K - 1, ij % K - 1
                    if use_odd and dj == 0:
                        plane, col0 = planes[1], 0
                    else:
                        plane, col0 = planes[0], 1 + dj
                    off = (1 + di) * GWP + col0
                    xsrc = (
                        plane[:, off : off + R * GWP]
                        .rearrange("p (r gc) -> p r gc", gc=GWP)
                        .rearrange("p r (g c) -> p r g c", c=WP)[:, :, :, 0:W]
                    )
                    wsrc = (
                        wc[:, ij * W : ij * W + R * WROW]
                        .rearrange("p (r s) -> p r s", s=WROW)[:, :, 0:W]
                        .unsqueeze(2)
                        .broadcast_to([128, R, G, W])
                    )
                    pr = pr_pool.tile([128, R * G * W], cdt, tag="pr")
                    prv = pr.rearrange("p (r g c) -> p r g c", g=G, c=W)
                    nc.vector.tensor_mul(out=prv, in0=xsrc, in1=wsrc)
                    for r in range(R):
                        nc.tensor.matmul(
                            ps_tiles[r // 2][
                                :, (r % 2) * G * W : (r % 2 + 1) * G * W
                            ],
                            idents[r % 2],
                            pr[:, r * G * W : (r + 1) * G * W],
                            start=(ij == 0),
                            stop=(ij == K * K - 1),
                        )

                # ---- evacuate PSUM and store (one 2-dim DMA)
                os_ = os_pool.tile([128, R * G * W], f32, tag="os")
                for t in range(4):
                    nc.scalar.copy(
                        out=os_[:, t * 2 * G * W : (t + 1) * 2 * G * W],
                        in_=ps_tiles[t],
                    )
                nc.sync.dma_start(out=o_t[:, y0 : y0 + R, :], in_=os_[:])

    nc.finalize()
    return nc


def _get(mode):
    if mode not in _cache:
        _cache[mode] = _build(mode)
    return _cache[mode]


def _swizzle_core(inp, wgt):
    # inp [2, 512, 64, 64] -> [128, H, G*W];  p = b*64+cw, free = (y, g, x)
    a = inp.reshape(NB, G, CW, H, W).transpose(0, 2, 3, 1, 4)
    a = np.ascontiguousarray(a).reshape(128, H, G * W)
    # wgt [2, 64, 9, 4096] -> [128, H, 9*W]; free = (y, ij, x)
    b = wgt.reshape(NB, CW, K * K, H, W).transpose(0, 1, 3, 2, 4)
    b = np.ascontiguousarray(b).reshape(128, H, K * K * W)
    return a, b


def _unswizzle_core(o):
    # [128, H, G*W] -> [2, 512, 64, 64]
    a = o.reshape(NB, CW, H, G, W).transpose(0, 3, 1, 2, 4)
    return np.ascontiguousarray(a).reshape(NB, C, H, W)


def kernel(input: np.ndarray, weight: np.ndarray) -> np.ndarray:
    from concourse.bass_utils import run_bass_kernel_spmd

    input = np.ascontiguousarray(input, dtype=np.float32)
    weight = np.ascontiguousarray(weight, dtype=np.float32)
    nc = _get(MODE)
    in_maps = []
    for i in range(NCORE):
        a, b = _swizzle_core(
            input[i * NB : (i + 1) * NB], weight[i * NB : (i + 1) * NB]
        )
        in_maps.append({"input_t": a, "weight_t": b})
    res = run_bass_kernel_spmd(nc, in_maps, core_ids=list(range(NCORE)))
    return np.concatenate(
        [_unswizzle_core(res.results[i]["out_t"]) for i in range(NCORE)], axis=0
    )
